# revision 27
# baseline (speedup 1.0000x reference)
"""Trainium2 Bass kernel for nn_Alignment_vector (sparse_attention).

Reference computation per batch b (B=128, Lq=128, Ls=256, d=1024, K=256):
  q = query * matrix                                  (Lq, d)
  A = context @ q.T                                   (Ls, Lq)
  A = leaky_relu(A, 0.1); A = A / ||A||_rows(q-axis)
  attn = softmax(smooth * A.T, axis=s)                (Lq, Ls)
  wc = attn @ context; wc = wc / ||wc||_rows(d-axis)  (Lq, d)
  sim = (query - wc)^2 @ W.T + b; out = sim / ||sim||_rows

Design notes (v2, ~150us vs 280us baseline):
  - All activation funcs ({Exp, Ln, Square, Copy}) live in ONE act table
    set (natural_log_exp_and_others) -> a single ACT_TABLE_LOAD (the
    act-table pass is steered via a scoped get_activation_tables patch in
    _build; default first-match placement thrashed 79 loads = 101us).
    sqrt/rsqrt are computed as exp(+-0.5*ln(x)); n2f must stay < 2^64 for
    the Ln table, hence the SIG down-scale in s_wsq/s_usub.
  - Softmax denominator and the wc/sim norm reciprocals cancel against the
    row l2norms downstream, so we never divide: tu = SIG*wcT - qT*SIG*||wc||
    and the bias is scaled by ||wc||^2 via a K=1 matmul (exact for any b).
  - Context ships in fp8 (both the [d,q]-transposed and natural copies) and
    q*matrix / exp(logits)/64 are cast to fp8 on the fly: mm1/mm2 in fp8
    cuts input DMA from 24MB to 16MB per core; all the fp8 quantization
    noise washes out through the l2norms/softmax (rel err 7.4e-3 vs 7.0e-3
    all-bf16). The /64 (via a memset bias AP on the Exp) keeps e^9 inside
    fp8 range; it is another per-row scale that cancels.
  - Batches emitted stage-interleaved in groups of 4 (next group's loads
    first) to keep PE continuously busy (p-state) and DMA saturated.
  - PSUM budget exactly 8 banks (bank-granular per buffer): psA 2x, psW 2x
    (evacuated by ACT Square -> tsq, freed at s_usub), psS 2x (psN|psB|psO
    packed in one [128,512] bank tile).
  - tensor_tensor_reduce (DVE ucode) wedges this runtime - use
    scalar_tensor_tensor (TensorScalarPtr) with accum_out instead; DVE
    instructions may read at most ONE operand from PSUM.
"""

import numpy as np
import ml_dtypes

import concourse.bass as bass
import concourse.bacc as bacc
import concourse.tile as tile
from concourse import mybir
from concourse.bass_utils import run_bass_kernel_spmd

B, LQ, LS, D, KS = 128, 128, 256, 1024, 256
NCORES = 8
BLOC = B // NCORES  # batches per core
DC = D // 128       # d chunks
GRP = 4             # batches per pipeline group
F32 = mybir.dt.float32
BF16 = mybir.dt.bfloat16
AF = mybir.ActivationFunctionType
ALU = mybir.AluOpType

MM_BF16 = True  # kept for test.py compat
FP8 = mybir.dt.float8e4
SIG = 2.0 ** -7  # wc down-scale; cancels in final l2norm (see s_wcopy)

_cache = {}


def _build(smooth: float, nb: int = BLOC, grp: int = GRP, taps: bool = False,
           no_ttr: bool = True, no_ln: bool = False, no_psumb: bool = False):
    key = (smooth, nb, grp, taps, no_ttr, no_ln, no_psumb)
    if key in _cache:
        return _cache[key]

    nc = bacc.Bacc("TRN2", debug=False)

    hqm = nc.dram_tensor("hqm", (nb, 128, 2048), BF16, kind="ExternalInput")
    hctx = nc.dram_tensor("hctx", (nb, 128, 4096), FP8, kind="ExternalInput")
    hw = nc.dram_tensor("hw", (128, DC, KS), BF16, kind="ExternalInput")
    hb = nc.dram_tensor("hb", (1, KS), BF16, kind="ExternalInput")
    hout = nc.dram_tensor("hout", (nb, LQ, KS), F32, kind="ExternalOutput")
    if taps:
        dt_n2a = nc.dram_tensor("dt_n2a", (nb, 128, 2), F32, kind="ExternalOutput")
        dt_te = nc.dram_tensor("dt_te", (nb, 128, 2, LQ), BF16, kind="ExternalOutput")
        dt_twc = nc.dram_tensor("dt_twc", (nb, 128, DC, LQ), BF16, kind="ExternalOutput")
        dt_tnw = nc.dram_tensor("dt_tnw", (nb, 1, LQ), BF16, kind="ExternalOutput")
        dt_tu = nc.dram_tensor("dt_tu", (nb, 128, DC, LQ), BF16, kind="ExternalOutput")
        dt_n2f = nc.dram_tensor("dt_n2f", (nb, 128, 1), F32, kind="ExternalOutput")

    inv_sm2 = 1.0 / (smooth * smooth)

    with tile.TileContext(nc) as tc:
        with (
            tc.tile_pool(name="const", bufs=1) as cpool,
            tc.tile_pool(name="inp", bufs=2 * grp) as ipool,
            tc.tile_pool(name="work", bufs=grp) as wpool,
            tc.tile_pool(name="ps_a", bufs=2, space="PSUM") as ps_a,
            tc.tile_pool(name="ps_w", bufs=2, space="PSUM") as ps_w,
            tc.tile_pool(name="ps_s", bufs=2, space="PSUM") as ps_s,
        ):
            tW = cpool.tile([128, DC, KS], BF16)
            nc.sync.dma_start(out=tW, in_=hw[:, :, :])
            tb = cpool.tile([1, KS], BF16)
            nc.sync.dma_start(out=tb, in_=hb[:, :])
            tones = cpool.tile([128, 1], BF16)
            nc.vector.memset(tones, 1.0)
            tones1 = cpool.tile([1, 128], BF16)
            nc.vector.memset(tones1, 1.0)
            # bias = -ln(64): te = exp(a*trs9)/64 fits fp8 (max 127)
            tbe = cpool.tile([128, 1], F32)
            nc.vector.memset(tbe, -4.1588830833596715)

            def s_load(v, bi):
                v["tqmT"] = ipool.tile([128, 2048], BF16, tag="tqmT", name="tqmT")
                v["tctx"] = ipool.tile([128, 4096], FP8, tag="tctx", name="tctx")
                nc.sync.dma_start(out=v["tqmT"], in_=hqm[bi])
                nc.sync.dma_start(out=v["tctx"], in_=hctx[bi])
                v["tqT"] = v["tqmT"][:, 0:1024].rearrange("p (j q) -> p j q", j=DC)
                v["tmT"] = v["tqmT"][:, 1024:2048].rearrange("p (j q) -> p j q", j=DC)
                v["tcT"] = v["tctx"][:, 0:2048].rearrange("p (j s) -> p j s", j=DC)
                v["tcn"] = v["tctx"][:, 2048:4096].rearrange("p (i d) -> p i d", i=2)

            def s_qm(v, bi):
                # q*matrix, transposed layout [d, q]
                v["tqm"] = wpool.tile([128, DC, LQ], FP8, tag="tqm", name="tqm")
                nc.vector.tensor_mul(
                    v["tqm"].rearrange("p j q -> p (j q)"),
                    v["tqT"].rearrange("p j q -> p (j q)"),
                    v["tmT"].rearrange("p j q -> p (j q)"),
                )

            def s_mm1(v, bi):
                # A[s, q] = sum_d context[s, d] qm[q, d]
                v["psA"] = ps_a.tile([128, 2, LQ], F32, tag="psA", name="psA")
                for i in range(2):
                    for j in range(DC):
                        nc.tensor.matmul(
                            v["psA"][:, i, :],
                            v["tcT"][:, j, 128 * i : 128 * i + 128],
                            v["tqm"][:, j, :],
                            start=(j == 0),
                            stop=(j == DC - 1),
                        )

            def s_prelu(v, bi):
                # leaky_relu(0.1) = max(x, 0.1x); 0.1x on ACT (Copy w/ scale),
                # max on DVE (only one PSUM operand allowed per DVE inst).
                # Frees psA afterwards.
                tal01 = wpool.tile([128, 2, LQ], BF16, tag="tal01")
                v["tal"] = wpool.tile([128, 2, LQ], BF16, tag="tal", name="tal")
                nc.scalar.activation(
                    tal01.rearrange("p a q -> p (a q)"),
                    v["psA"].rearrange("p a q -> p (a q)"),
                    AF.Copy,
                    scale=0.1,
                )
                nc.vector.tensor_max(
                    v["tal"].rearrange("p a q -> p (a q)"),
                    v["psA"].rearrange("p a q -> p (a q)"),
                    tal01.rearrange("p a q -> p (a q)"),
                )

            def s_n2a(v, bi):
                # n2A = sum_q leaky^2 / smooth^2, fused square+reduce on DVE:
                # (tal * inv_sm2) * tal with accum_out (one op per s-tile)
                v["tn2A"] = wpool.tile([128, 2], F32, tag="tn2A", name="tn2A")
                scrA = wpool.tile([128, 2, LQ], BF16, tag="scrA")
                for i in range(2):
                    nc.vector.scalar_tensor_tensor(
                        scrA[:, i, :],
                        v["tal"][:, i, :],
                        inv_sm2,
                        v["tal"][:, i, :],
                        ALU.mult,
                        ALU.mult,
                        accum_out=v["tn2A"][:, i : i + 1],
                    )

            def s_trs(v, bi):
                # trs9 = smooth / ||leaky_row|| = exp(-0.5 * ln(n2A))
                tlnA = wpool.tile([128, 2], F32, tag="tlnA")
                v["trs9"] = wpool.tile([128, 2], F32, tag="trs9", name="trs9")
                insc = 1.0
                if no_ln:
                    nc.scalar.activation(tlnA, v["tn2A"], AF.Sqrt, scale=insc)
                    nc.vector.reciprocal(v["trs9"], tlnA)
                else:
                    nc.scalar.activation(tlnA, v["tn2A"], AF.Ln, scale=insc)
                    nc.scalar.activation(v["trs9"], tlnA, AF.Exp, scale=-0.5)

            def s_exp(v, bi):
                # te = exp(a * trs9 - ln 64) in fp8; the 1/64 (and fp8
                # context) scales wc per-row, which cancels downstream
                v["te"] = wpool.tile([128, 2, LQ], FP8, tag="te", name="te")
                for i in range(2):
                    nc.scalar.activation(
                        v["te"][:, i, :],
                        v["tal"][:, i, :],
                        AF.Exp,
                        bias=tbe[:, 0:1],
                        scale=v["trs9"][:, i : i + 1],
                    )

            def s_mm2(v, bi):
                # wcT[d, q] = sum_s context[s, d] e[s, q]
                v["psW"] = ps_w.tile([128, DC, LQ], F32, tag="psW", name="psW")
                for j in range(DC):
                    for i in range(2):
                        nc.tensor.matmul(
                            v["psW"][:, j, :],
                            v["tcn"][:, i, 128 * j : 128 * j + 128],
                            v["te"][:, i, :],
                            start=(i == 0),
                            stop=(i == 1),
                        )

            def s_wsq(v, bi):
                # tsq = (SIG*wc)^2 straight from PSUM (scale inside Square),
                # in halves so wones can chase the first half.
                # SIG keeps downstream magnitudes in the act-table Ln range
                # (n2f reaches ~5e19 > 2^64 unscaled); every psO row picks up
                # a consistent SIG^2 which the final l2norm cancels.
                v["tsq"] = wpool.tile([128, DC, LQ], BF16, tag="tsq", name="tsq")
                nc.scalar.activation(
                    v["tsq"].rearrange("p j q -> p (j q)"),
                    v["psW"].rearrange("p j q -> p (j q)"),
                    AF.Square,
                    scale=SIG,
                )

            def s_wones(v, bi):
                # n2w[1, q] = sum_d wc^2 via ones-matmul partition reduction
                # psN/psB/psO share one PSUM bank tile [128, 512]
                psS = ps_s.tile([128, 512], F32, tag="psS", name="psS")
                v["psN"] = psS[0:1, 0:LQ]
                v["psB"] = psS[:, LQ : 2 * LQ]
                v["psO"] = psS[:, 2 * LQ : 2 * LQ + KS]
                for j in range(DC):
                    nc.tensor.matmul(
                        v["psN"],
                        tones,
                        v["tsq"][:, j, :],
                        start=(j == 0),
                        stop=(j == DC - 1),
                    )

            def s_wnorm(v, bi):
                # tnw = ||wc|| = exp(0.5 * ln(n2w)) ; tnwsq = ||wc||^2
                tlnw = wpool.tile([1, LQ], F32, tag="tlnw")
                v["tnw"] = wpool.tile([1, LQ], BF16, tag="tnw", name="tnw")
                if no_ln:
                    nc.scalar.activation(v["tnw"], v["psN"], AF.Sqrt)
                else:
                    nc.scalar.activation(tlnw, v["psN"], AF.Ln)
                    nc.scalar.activation(v["tnw"], tlnw, AF.Exp, scale=0.5)

            def s_wnsq(v, bi):
                v["tnwsq"] = wpool.tile([1, LQ], BF16, tag="tnwsq", name="tnwsq")
                nc.vector.tensor_mul(v["tnwsq"], v["tnw"], v["tnw"])

            def s_bcast(v, bi):
                # broadcast ||wc|| across partitions with K=1 matmul
                nc.tensor.matmul(v["psB"], tones1, v["tnw"], start=True, stop=True)

            def s_umul(v, bi):
                # u = qT * ||wc|| (broadcast along d-chunks via stride-0),
                # halves so usub/simsq/mm3 can chase
                src = v["psB"]
                src_b = bass.AP(
                    tensor=src.tensor,
                    offset=src.offset,
                    ap=[list(src.ap[0]), [0, DC], list(src.ap[1])],
                )
                v["tum"] = wpool.tile([128, DC, LQ], BF16, tag="tum", name="tum")
                nc.vector.tensor_mul(v["tum"], v["tqT"], src_b)

            def s_usub(v, bi):
                # tu = SIG*wcT - qT*SIG*||wc|| = -u; sign cancels in square.
                # Frees psW (its last reader).
                v["tu"] = wpool.tile([128, DC, LQ], BF16, tag="tu", name="tu")
                nc.vector.scalar_tensor_tensor(
                    v["tu"].rearrange("p j q -> p (j q)"),
                    v["psW"].rearrange("p j q -> p (j q)"),
                    SIG,
                    v["tum"].rearrange("p j q -> p (j q)"),
                    ALU.mult,
                    ALU.subtract,
                )

            def s_simsq(v, bi):
                v["tsim"] = wpool.tile([128, DC, LQ], BF16, tag="tsim", name="tsim")
                nc.gpsimd.tensor_mul(
                    v["tsim"].rearrange("p j q -> p (j q)"),
                    v["tu"].rearrange("p j q -> p (j q)"),
                    v["tu"].rearrange("p j q -> p (j q)"),
                )

            def s_mm3(v, bi):
                # out[q, k] = sum_d sim[q, d] W[k, d] + ||wc||^2 * b
                for j in range(DC):
                    nc.tensor.matmul(
                        v["psO"],
                        v["tsim"][:, j, :],
                        tW[:, j, :],
                        start=(j == 0),
                        stop=False,
                    )
                nc.tensor.matmul(v["psO"], v["tnwsq"], tb, start=False, stop=True)

            def s_fsq(v, bi):
                scrO = wpool.tile([128, KS], BF16, tag="scrO")
                v["tn2f"] = wpool.tile([128, 1], F32, tag="tn2f", name="tn2f")
                nc.scalar.activation(
                    scrO, v["psO"], AF.Square, accum_out=v["tn2f"]
                )

            def s_taps(v, bi):
                if not taps:
                    return
                nc.sync.dma_start(out=dt_n2a[bi], in_=v["tn2A"])
                nc.sync.dma_start(out=dt_te[bi], in_=v["te"])
                nc.sync.dma_start(out=dt_tnw[bi], in_=v["tnw"])

            def s_taps2(v, bi):
                if not taps:
                    return
                nc.sync.dma_start(out=dt_n2f[bi], in_=v["tn2f"])

            def s_fnorm(v, bi):
                tlnf = wpool.tile([128, 1], F32, tag="tlnf")
                v["trf"] = wpool.tile([128, 1], F32, tag="trf", name="trf")
                if no_ln:
                    nc.scalar.activation(tlnf, v["tn2f"], AF.Sqrt)
                    nc.vector.reciprocal(v["trf"], tlnf)
                else:
                    nc.scalar.activation(tlnf, v["tn2f"], AF.Ln)
                    nc.scalar.activation(v["trf"], tlnf, AF.Exp, scale=-0.5)

            def s_fout(v, bi):
                tout = wpool.tile([128, KS], F32, tag="tout")
                nc.vector.tensor_scalar_mul(tout, v["psO"], v["trf"][:, 0:1])
                nc.sync.dma_start(out=hout[bi], in_=tout)

            stages = [
                s_qm, s_mm1, s_prelu, s_n2a, s_trs, s_exp, s_mm2,
                s_wsq, s_wones, s_wnorm, s_wnsq, s_bcast, s_umul, s_usub,
                s_simsq, s_taps, s_mm3, s_fsq, s_taps2, s_fnorm, s_fout,
            ]

            groups = [
                [({}, b0 + k) for k in range(grp)]
                for b0 in range(0, nb, grp)
            ]
            # prime: loads for group 0
            for v, bi in groups[0]:
                s_load(v, bi)
            for gi, grp in enumerate(groups):
                # kick next group's loads first so DMA stays saturated
                if gi + 1 < len(groups):
                    for v, bi in groups[gi + 1]:
                        s_load(v, bi)
                for stage in stages:
                    for v, bi in grp:
                        stage(v, bi)

    # The act-table-load pass assigns each activation the FIRST table set
    # containing its function; Exp/Copy/Square first-match set 0 while Ln
    # matches set 5, so mixed use thrashes ACT_TABLE_LOADs (1.3us each).
    # Narrow the tables (set indices preserved) so {Exp, Ln, Square, Copy}
    # first-match only in natural_log_exp_and_others -> exactly one load.
    _mine = {AF.Exp, AF.Ln, AF.Square, AF.Copy, AF.Identity}
    _orig_tables = bacc.get_activation_tables
    def _narrowed(arch):
        full = _orig_tables(arch)
        return {
            name: (set(fns) if name == "natural_log_exp_and_others"
                   else set(fns) - _mine)
            for name, fns in full.items()
        }
    bacc.get_activation_tables = _narrowed
    try:
        nc.compile()
    finally:
        bacc.get_activation_tables = _orig_tables
    _cache[key] = nc
    return nc


def _prep(query, context, matrix, W, b):
    bf = ml_dtypes.bfloat16
    f8 = mybir.dt.np(FP8)
    # [b, p, j, q] = x[b, q, 128j+p]
    qT = query.reshape(B, LQ, DC, 128).transpose(0, 3, 2, 1).reshape(B, 128, 1024)
    mT = matrix.reshape(B, LQ, DC, 128).transpose(0, 3, 2, 1).reshape(B, 128, 1024)
    hqm = np.ascontiguousarray(
        np.concatenate([qT.astype(bf), mT.astype(bf)], axis=2)
    )
    # [b, p, j, s] = context[b, s, 128j+p]
    cT = context.reshape(B, LS, DC, 128).transpose(0, 3, 2, 1).reshape(B, 128, 2048)
    # [b, p, i, d] = context[b, 128i+p, d]
    cn = context.reshape(B, 2, 128, D).transpose(0, 2, 1, 3).reshape(B, 128, 2048)
    hctx = np.ascontiguousarray(
        np.concatenate([cT.astype(f8), cn.astype(f8)], axis=2)
    )
    # [p, j, k] = W[k, 128j+p]
    hw = np.ascontiguousarray(W.reshape(KS, DC, 128).transpose(2, 1, 0)).astype(bf)
    hb = np.ascontiguousarray(b.reshape(1, KS)).astype(bf)
    return hqm, hctx, hw, hb


def kernel(query, context, matrix, W, b, smooth, _trace=False):
    query = np.asarray(query, dtype=np.float32)
    context = np.asarray(context, dtype=np.float32)
    matrix = np.asarray(matrix, dtype=np.float32)
    W = np.asarray(W, dtype=np.float32)
    b = np.asarray(b, dtype=np.float32)

    nc = _build(float(smooth))
    hqm, hctx, hw, hb = _prep(query, context, matrix, W, b)

    in_maps = []
    for c in range(NCORES):
        sl = slice(c * BLOC, (c + 1) * BLOC)
        in_maps.append({"hqm": hqm[sl], "hctx": hctx[sl], "hw": hw, "hb": hb})

    res = run_bass_kernel_spmd(
        nc, in_maps, core_ids=list(range(NCORES)), trace=_trace
    )
    out = np.concatenate([r["hout"] for r in res.results], axis=0)
    out = np.ascontiguousarray(out.astype(np.float32))
    if _trace:
        return out, res
    return out


# revision 28
# speedup vs baseline: 1.0725x; 1.0725x over previous
"""Trainium2 Bass kernel for nn_Alignment_vector (sparse_attention).

Reference computation per batch b (B=128, Lq=128, Ls=256, d=1024, K=256):
  q = query * matrix                                  (Lq, d)
  A = context @ q.T                                   (Ls, Lq)
  A = leaky_relu(A, 0.1); A = A / ||A||_rows(q-axis)
  attn = softmax(smooth * A.T, axis=s)                (Lq, Ls)
  wc = attn @ context; wc = wc / ||wc||_rows(d-axis)  (Lq, d)
  sim = (query - wc)^2 @ W.T + b; out = sim / ||sim||_rows

Design notes (v2, ~150us vs 280us baseline):
  - All activation funcs ({Exp, Ln, Square, Copy}) live in ONE act table
    set (natural_log_exp_and_others) -> a single ACT_TABLE_LOAD (the
    act-table pass is steered via a scoped get_activation_tables patch in
    _build; default first-match placement thrashed 79 loads = 101us).
    sqrt/rsqrt are computed as exp(+-0.5*ln(x)); n2f must stay < 2^64 for
    the Ln table, hence the SIG down-scale in s_wsq/s_usub.
  - Softmax denominator and the wc/sim norm reciprocals cancel against the
    row l2norms downstream, so we never divide: tu = SIG*wcT - qT*SIG*||wc||
    and the bias is scaled by ||wc||^2 via a K=1 matmul (exact for any b).
  - Context ships in fp8 (both the [d,q]-transposed and natural copies) and
    q*matrix / exp(logits)/64 are cast to fp8 on the fly: mm1/mm2 in fp8
    cuts input DMA from 24MB to 16MB per core; all the fp8 quantization
    noise washes out through the l2norms/softmax (rel err 7.4e-3 vs 7.0e-3
    all-bf16). The /64 (via a memset bias AP on the Exp) keeps e^9 inside
    fp8 range; it is another per-row scale that cancels.
  - Batches emitted stage-interleaved in groups of 4 (next group's loads
    first) to keep PE continuously busy (p-state) and DMA saturated.
  - PSUM budget exactly 8 banks (bank-granular per buffer): psA 2x, psW 2x
    (evacuated by ACT Square -> tsq, freed at s_usub), psS 2x (psN|psB|psO
    packed in one [128,512] bank tile).
  - tensor_tensor_reduce (DVE ucode) wedges this runtime - use
    scalar_tensor_tensor (TensorScalarPtr) with accum_out instead; DVE
    instructions may read at most ONE operand from PSUM.
"""

import numpy as np
import ml_dtypes

import concourse.bass as bass
import concourse.bacc as bacc
import concourse.tile as tile
from concourse import mybir
from concourse.bass_utils import run_bass_kernel_spmd

B, LQ, LS, D, KS = 128, 128, 256, 1024, 256
NCORES = 8
BLOC = B // NCORES  # batches per core
DC = D // 128       # d chunks
GRP = 4             # batches per pipeline group
F32 = mybir.dt.float32
BF16 = mybir.dt.bfloat16
AF = mybir.ActivationFunctionType
ALU = mybir.AluOpType

MM_BF16 = True  # kept for test.py compat
FP8 = mybir.dt.float8e4
SIG = 2.0 ** -7  # wc down-scale; cancels in final l2norm (see s_wcopy)

_cache = {}


def _build(smooth: float, nb: int = BLOC, grp: int = GRP, taps: bool = False,
           no_ttr: bool = True, no_ln: bool = False, no_psumb: bool = False):
    key = (smooth, nb, grp, taps, no_ttr, no_ln, no_psumb)
    if key in _cache:
        return _cache[key]

    nc = bacc.Bacc("TRN2", debug=False)

    hqm = nc.dram_tensor("hqm", (nb, 128, 2048), BF16, kind="ExternalInput")
    hctx = nc.dram_tensor("hctx", (nb, 128, 4096), FP8, kind="ExternalInput")
    hw = nc.dram_tensor("hw", (128, DC, KS), BF16, kind="ExternalInput")
    hb = nc.dram_tensor("hb", (1, KS), BF16, kind="ExternalInput")
    hout = nc.dram_tensor("hout", (nb, LQ, KS), F32, kind="ExternalOutput")
    if taps:
        dt_n2a = nc.dram_tensor("dt_n2a", (nb, 128, 2), F32, kind="ExternalOutput")
        dt_te = nc.dram_tensor("dt_te", (nb, 128, 2, LQ), BF16, kind="ExternalOutput")
        dt_twc = nc.dram_tensor("dt_twc", (nb, 128, DC, LQ), BF16, kind="ExternalOutput")
        dt_tnw = nc.dram_tensor("dt_tnw", (nb, 1, LQ), BF16, kind="ExternalOutput")
        dt_tu = nc.dram_tensor("dt_tu", (nb, 128, DC, LQ), BF16, kind="ExternalOutput")
        dt_n2f = nc.dram_tensor("dt_n2f", (nb, 128, 1), F32, kind="ExternalOutput")

    inv_sm2 = 1.0 / (smooth * smooth)

    with tile.TileContext(nc) as tc:
        with (
            tc.tile_pool(name="const", bufs=1) as cpool,
            tc.tile_pool(name="inp", bufs=2 * grp) as ipool,
            tc.tile_pool(name="work", bufs=grp) as wpool,
            tc.tile_pool(name="ps_a", bufs=2, space="PSUM") as ps_a,
            tc.tile_pool(name="ps_w", bufs=2, space="PSUM") as ps_w,
            tc.tile_pool(name="ps_s", bufs=2, space="PSUM") as ps_s,
        ):
            tW = cpool.tile([128, DC, KS], BF16)
            nc.sync.dma_start(out=tW, in_=hw[:, :, :])
            tb = cpool.tile([1, KS], BF16)
            nc.sync.dma_start(out=tb, in_=hb[:, :])
            tones = cpool.tile([128, 1], BF16)
            nc.vector.memset(tones, 1.0)
            tones1 = cpool.tile([1, 128], BF16)
            nc.vector.memset(tones1, 1.0)
            # bias = -ln(64): te = exp(a*trs9)/64 fits fp8 (max 127)
            tbe = cpool.tile([128, 1], F32)
            nc.vector.memset(tbe, -4.1588830833596715)

            def s_load(v, bi):
                v["tqmT"] = ipool.tile([128, 2048], BF16, tag="tqmT", name="tqmT")
                v["tctx"] = ipool.tile([128, 4096], FP8, tag="tctx", name="tctx")
                nc.sync.dma_start(out=v["tqmT"], in_=hqm[bi])
                nc.sync.dma_start(out=v["tctx"], in_=hctx[bi])
                v["tqT"] = v["tqmT"][:, 0:1024].rearrange("p (j q) -> p j q", j=DC)
                v["tmT"] = v["tqmT"][:, 1024:2048].rearrange("p (j q) -> p j q", j=DC)
                v["tcT"] = v["tctx"][:, 0:2048].rearrange("p (j s) -> p j s", j=DC)
                v["tcn"] = v["tctx"][:, 2048:4096].rearrange("p (i d) -> p i d", i=2)

            def s_qm(v, bi):
                # q*matrix, transposed layout [d, q]
                v["tqm"] = wpool.tile([128, DC, LQ], FP8, tag="tqm", name="tqm")
                nc.vector.tensor_mul(
                    v["tqm"].rearrange("p j q -> p (j q)"),
                    v["tqT"].rearrange("p j q -> p (j q)"),
                    v["tmT"].rearrange("p j q -> p (j q)"),
                )

            def s_mm1(v, bi):
                # A[s, q] = sum_d context[s, d] qm[q, d]
                v["psA"] = ps_a.tile([128, 2, LQ], F32, tag="psA", name="psA")
                for i in range(2):
                    for j in range(DC):
                        nc.tensor.matmul(
                            v["psA"][:, i, :],
                            v["tcT"][:, j, 128 * i : 128 * i + 128],
                            v["tqm"][:, j, :],
                            start=(j == 0),
                            stop=(j == DC - 1),
                        )

            def s_prelu(v, bi):
                # leaky_relu(0.1) = max(x, 0.1x); 0.1x on ACT (Copy w/ scale),
                # max on DVE (only one PSUM operand allowed per DVE inst).
                # Frees psA afterwards.
                tal01 = wpool.tile([128, 2, LQ], BF16, tag="tal01")
                v["tal"] = wpool.tile([128, 2, LQ], BF16, tag="tal", name="tal")
                nc.scalar.activation(
                    tal01.rearrange("p a q -> p (a q)"),
                    v["psA"].rearrange("p a q -> p (a q)"),
                    AF.Copy,
                    scale=0.1,
                )
                nc.vector.tensor_max(
                    v["tal"].rearrange("p a q -> p (a q)"),
                    v["psA"].rearrange("p a q -> p (a q)"),
                    tal01.rearrange("p a q -> p (a q)"),
                )

            def s_n2a(v, bi):
                # n2A = sum_q leaky^2 / smooth^2, fused square+reduce on DVE:
                # (tal * inv_sm2) * tal with accum_out (one op per s-tile)
                v["tn2A"] = wpool.tile([128, 2], F32, tag="tn2A", name="tn2A")
                scrA = wpool.tile([128, 2, LQ], BF16, tag="scrA")
                for i in range(2):
                    nc.vector.scalar_tensor_tensor(
                        scrA[:, i, :],
                        v["tal"][:, i, :],
                        inv_sm2,
                        v["tal"][:, i, :],
                        ALU.mult,
                        ALU.mult,
                        accum_out=v["tn2A"][:, i : i + 1],
                    )

            def s_trs(v, bi):
                # trs9 = smooth / ||leaky_row|| = exp(-0.5 * ln(n2A))
                tlnA = wpool.tile([128, 2], F32, tag="tlnA")
                v["trs9"] = wpool.tile([128, 2], F32, tag="trs9", name="trs9")
                insc = 1.0
                if no_ln:
                    nc.scalar.activation(tlnA, v["tn2A"], AF.Sqrt, scale=insc)
                    nc.vector.reciprocal(v["trs9"], tlnA)
                else:
                    nc.scalar.activation(tlnA, v["tn2A"], AF.Ln, scale=insc)
                    nc.scalar.activation(v["trs9"], tlnA, AF.Exp, scale=-0.5)

            def s_exp(v, bi):
                # te = exp(a * trs9 - ln 64) in fp8; the 1/64 (and fp8
                # context) scales wc per-row, which cancels downstream
                v["te"] = wpool.tile([128, 2, LQ], FP8, tag="te", name="te")
                for i in range(2):
                    nc.scalar.activation(
                        v["te"][:, i, :],
                        v["tal"][:, i, :],
                        AF.Exp,
                        bias=tbe[:, 0:1],
                        scale=v["trs9"][:, i : i + 1],
                    )

            def s_mm2(v, bi):
                # wcT[d, q] = sum_s context[s, d] e[s, q]
                v["psW"] = ps_w.tile([128, DC, LQ], F32, tag="psW", name="psW")
                for j in range(DC):
                    for i in range(2):
                        nc.tensor.matmul(
                            v["psW"][:, j, :],
                            v["tcn"][:, i, 128 * j : 128 * j + 128],
                            v["te"][:, i, :],
                            start=(i == 0),
                            stop=(i == 1),
                        )

            def s_wsq(v, bi):
                # tsq = (SIG*wc)^2 straight from PSUM (scale inside Square),
                # in halves so wones can chase the first half.
                # SIG keeps downstream magnitudes in the act-table Ln range
                # (n2f reaches ~5e19 > 2^64 unscaled); every psO row picks up
                # a consistent SIG^2 which the final l2norm cancels.
                v["tsq"] = wpool.tile([128, DC, LQ], BF16, tag="tsq", name="tsq")
                nc.scalar.activation(
                    v["tsq"].rearrange("p j q -> p (j q)"),
                    v["psW"].rearrange("p j q -> p (j q)"),
                    AF.Square,
                    scale=SIG,
                )

            def s_wones(v, bi):
                # n2w[1, q] = sum_d wc^2 via ones-matmul partition reduction
                # psN/psB/psO share one PSUM bank tile [128, 512]
                psS = ps_s.tile([128, 512], F32, tag="psS", name="psS")
                v["psN"] = psS[0:1, 0:LQ]
                v["psB"] = psS[:, LQ : 2 * LQ]
                v["psO"] = psS[:, 2 * LQ : 2 * LQ + KS]
                for j in range(DC):
                    nc.tensor.matmul(
                        v["psN"],
                        tones,
                        v["tsq"][:, j, :],
                        start=(j == 0),
                        stop=(j == DC - 1),
                    )

            def s_wnorm(v, bi):
                # tnw = ||wc|| = exp(0.5 * ln(n2w)) ; tnwsq = ||wc||^2
                tlnw = wpool.tile([1, LQ], F32, tag="tlnw")
                v["tnw"] = wpool.tile([1, LQ], BF16, tag="tnw", name="tnw")
                if no_ln:
                    nc.scalar.activation(v["tnw"], v["psN"], AF.Sqrt)
                else:
                    nc.scalar.activation(tlnw, v["psN"], AF.Ln)
                    nc.scalar.activation(v["tnw"], tlnw, AF.Exp, scale=0.5)

            def s_wnsq(v, bi):
                v["tnwsq"] = wpool.tile([1, LQ], BF16, tag="tnwsq", name="tnwsq")
                nc.vector.tensor_mul(v["tnwsq"], v["tnw"], v["tnw"])

            def s_bcast(v, bi):
                # broadcast ||wc|| across partitions with K=1 matmul
                nc.tensor.matmul(v["psB"], tones1, v["tnw"], start=True, stop=True)

            def s_umul(v, bi):
                # u = qT * ||wc|| (broadcast along d-chunks via stride-0),
                # halves so usub/simsq/mm3 can chase
                src = v["psB"]
                src_b = bass.AP(
                    tensor=src.tensor,
                    offset=src.offset,
                    ap=[list(src.ap[0]), [0, DC], list(src.ap[1])],
                )
                v["tum"] = wpool.tile([128, DC, LQ], BF16, tag="tum", name="tum")
                nc.vector.tensor_mul(v["tum"], v["tqT"], src_b)

            def s_usub(v, bi):
                # tu = SIG*wcT - qT*SIG*||wc|| = -u; sign cancels in square.
                # Frees psW (its last reader).
                v["tu"] = wpool.tile([128, DC, LQ], BF16, tag="tu", name="tu")
                nc.vector.scalar_tensor_tensor(
                    v["tu"].rearrange("p j q -> p (j q)"),
                    v["psW"].rearrange("p j q -> p (j q)"),
                    SIG,
                    v["tum"].rearrange("p j q -> p (j q)"),
                    ALU.mult,
                    ALU.subtract,
                )

            def s_simsq(v, bi):
                v["tsim"] = wpool.tile([128, DC, LQ], BF16, tag="tsim", name="tsim")
                nc.gpsimd.tensor_mul(
                    v["tsim"].rearrange("p j q -> p (j q)"),
                    v["tu"].rearrange("p j q -> p (j q)"),
                    v["tu"].rearrange("p j q -> p (j q)"),
                )

            def s_mm3(v, bi):
                # out[q, k] = sum_d sim[q, d] W[k, d] + ||wc||^2 * b
                for j in range(DC):
                    nc.tensor.matmul(
                        v["psO"],
                        v["tsim"][:, j, :],
                        tW[:, j, :],
                        start=(j == 0),
                        stop=False,
                    )
                nc.tensor.matmul(v["psO"], v["tnwsq"], tb, start=False, stop=True)

            def s_fsq(v, bi):
                scrO = wpool.tile([128, KS], BF16, tag="scrO")
                v["tn2f"] = wpool.tile([128, 1], F32, tag="tn2f", name="tn2f")
                nc.scalar.activation(
                    scrO, v["psO"], AF.Square, accum_out=v["tn2f"]
                )

            def s_taps(v, bi):
                if not taps:
                    return
                nc.sync.dma_start(out=dt_n2a[bi], in_=v["tn2A"])
                nc.sync.dma_start(out=dt_te[bi], in_=v["te"])
                nc.sync.dma_start(out=dt_tnw[bi], in_=v["tnw"])

            def s_taps2(v, bi):
                if not taps:
                    return
                nc.sync.dma_start(out=dt_n2f[bi], in_=v["tn2f"])

            def s_fnorm(v, bi):
                tlnf = wpool.tile([128, 1], F32, tag="tlnf")
                v["trf"] = wpool.tile([128, 1], F32, tag="trf", name="trf")
                if no_ln:
                    nc.scalar.activation(tlnf, v["tn2f"], AF.Sqrt)
                    nc.vector.reciprocal(v["trf"], tlnf)
                else:
                    nc.scalar.activation(tlnf, v["tn2f"], AF.Ln)
                    nc.scalar.activation(v["trf"], tlnf, AF.Exp, scale=-0.5)

            def s_fout(v, bi):
                tout = wpool.tile([128, KS], F32, tag="tout")
                nc.vector.tensor_scalar_mul(tout, v["psO"], v["trf"][:, 0:1])
                nc.sync.dma_start(out=hout[bi], in_=tout)

            def s_utail(v, bi):
                # per-batch back-to-back: keeps usub/simsq from queueing
                # behind other batches' umuls (DVE/GPS convoy effect)
                s_umul(v, bi)
                s_usub(v, bi)
                s_simsq(v, bi)

            def s_otail(v, bi):
                s_mm3(v, bi)
                s_fsq(v, bi)
                s_fnorm(v, bi)
                s_fout(v, bi)

            stages = [
                s_qm, s_mm1, s_prelu, s_n2a, s_trs, s_exp, s_mm2,
                s_wsq, s_wones, s_wnorm, s_wnsq, s_bcast, s_utail,
                s_taps, s_otail, s_taps2,
            ]

            groups = [
                [({}, b0 + k) for k in range(grp)]
                for b0 in range(0, nb, grp)
            ]
            # prime: loads for group 0
            for v, bi in groups[0]:
                s_load(v, bi)
            for gi, grp in enumerate(groups):
                # kick next group's loads first so DMA stays saturated
                if gi + 1 < len(groups):
                    for v, bi in groups[gi + 1]:
                        s_load(v, bi)
                for stage in stages:
                    for v, bi in grp:
                        stage(v, bi)

    # The act-table-load pass assigns each activation the FIRST table set
    # containing its function; Exp/Copy/Square first-match set 0 while Ln
    # matches set 5, so mixed use thrashes ACT_TABLE_LOADs (1.3us each).
    # Narrow the tables (set indices preserved) so {Exp, Ln, Square, Copy}
    # first-match only in natural_log_exp_and_others -> exactly one load.
    _mine = {AF.Exp, AF.Ln, AF.Square, AF.Copy, AF.Identity}
    _orig_tables = bacc.get_activation_tables
    def _narrowed(arch):
        full = _orig_tables(arch)
        return {
            name: (set(fns) if name == "natural_log_exp_and_others"
                   else set(fns) - _mine)
            for name, fns in full.items()
        }
    bacc.get_activation_tables = _narrowed
    try:
        nc.compile()
    finally:
        bacc.get_activation_tables = _orig_tables
    _cache[key] = nc
    return nc


def _prep(query, context, matrix, W, b):
    bf = ml_dtypes.bfloat16
    f8 = mybir.dt.np(FP8)
    # [b, p, j, q] = x[b, q, 128j+p]
    qT = query.reshape(B, LQ, DC, 128).transpose(0, 3, 2, 1).reshape(B, 128, 1024)
    mT = matrix.reshape(B, LQ, DC, 128).transpose(0, 3, 2, 1).reshape(B, 128, 1024)
    hqm = np.ascontiguousarray(
        np.concatenate([qT.astype(bf), mT.astype(bf)], axis=2)
    )
    # [b, p, j, s] = context[b, s, 128j+p]
    cT = context.reshape(B, LS, DC, 128).transpose(0, 3, 2, 1).reshape(B, 128, 2048)
    # [b, p, i, d] = context[b, 128i+p, d]
    cn = context.reshape(B, 2, 128, D).transpose(0, 2, 1, 3).reshape(B, 128, 2048)
    hctx = np.ascontiguousarray(
        np.concatenate([cT.astype(f8), cn.astype(f8)], axis=2)
    )
    # [p, j, k] = W[k, 128j+p]
    hw = np.ascontiguousarray(W.reshape(KS, DC, 128).transpose(2, 1, 0)).astype(bf)
    hb = np.ascontiguousarray(b.reshape(1, KS)).astype(bf)
    return hqm, hctx, hw, hb


def kernel(query, context, matrix, W, b, smooth, _trace=False):
    query = np.asarray(query, dtype=np.float32)
    context = np.asarray(context, dtype=np.float32)
    matrix = np.asarray(matrix, dtype=np.float32)
    W = np.asarray(W, dtype=np.float32)
    b = np.asarray(b, dtype=np.float32)

    nc = _build(float(smooth))
    hqm, hctx, hw, hb = _prep(query, context, matrix, W, b)

    in_maps = []
    for c in range(NCORES):
        sl = slice(c * BLOC, (c + 1) * BLOC)
        in_maps.append({"hqm": hqm[sl], "hctx": hctx[sl], "hw": hw, "hb": hb})

    res = run_bass_kernel_spmd(
        nc, in_maps, core_ids=list(range(NCORES)), trace=_trace
    )
    out = np.concatenate([r["hout"] for r in res.results], axis=0)
    out = np.ascontiguousarray(out.astype(np.float32))
    if _trace:
        return out, res
    return out
